# revision 39
# baseline (speedup 1.0000x reference)
"""KANConv2D Trainium2 kernel (8 NeuronCores, data-parallel over batch).

Math: out = conv(x, kernel) + exp(-gamma * d) + bias, where
  d[n,f]  = pn[n] + cn[f] - 2*pc[n,f]
  pc      = patches(x) @ control_points
  pn[n]   = sum of x^2 over the 3x3xC patch
  gamma   = 1 / (2 * mean(d))            (global mean -> AllReduce)

Device strategy per core (4 images), v2:
  - q := pc - pn/2 runs in fp8e4m3 with DoubleRow matmuls: rhs partitions
    hold [x8; x8^2] stored as THREE column-shifted copies with row stride
    exactly 64, so a block's 8x64 window is one contiguous 512-run and the
    DoubleRow ifmap is the required 3-dim [128, 2, 512] AP whose pair dim
    strides between copies/rows (both multiples of 16). Each DoubleRow
    covers two of the 9 taps -> 5 matmuls per 512-pixel block.
  - conv runs in bf16 with K=128 tap pairing: SBUF tile xc = [x | x
    shifted left one column], so taps (kh,0)+(kh,1) fuse into one K=128
    matmul and taps (kh,2) run zero-padded to K=128 (full-width matmuls
    hold the PE boost clock; K=64 decays to mid p-state).
  - conv results stay in SBUF (no DRAM scratch roundtrip).
  - gamma: the global sum is split in two halves, each AllReduced as soon
    as its half of q finishes; the first collective absorbs CC-core setup
    + cross-core skew so the second (which gates the epilogue) is short.
  - epilogue (exp + add + store) is interleaved per block with the conv
    matmul groups: ACT does exp, DVE drains conv PSUM, GPSIMD+DVE split
    the final add, so the post-PE tail is only a few microseconds.
"""

import os
import sys

import numpy as np

for _p in ("/opt/trn_rl_repo", "/root/.axon_site/_ro/trn_rl_repo"):
    if os.path.isdir(_p) and _p not in sys.path:
        sys.path.insert(0, _p)

import ml_dtypes

import concourse.bacc as bacc
import concourse.bass_utils as _bu
import concourse.tile as tile
from concourse import mybir
from concourse.ap import AP
from concourse.bass_utils import run_bass_kernel_spmd


def _ensure_ntff_hook():
    """bass_utils imports antenv.axon_hooks when tracing under axon; this
    image's antenv lacks that module. Provide it and install the ctypes
    NTFF hook so BASS_TRACE=1 yields exec_time_ns."""
    import types
    try:
        from antenv.axon_hooks import get_axon_ntff_profile_hook  # noqa: F401
        return
    except ImportError:
        pass
    try:
        import antenv
        mod = types.ModuleType("antenv.axon_hooks")
        _state = {"hook": None}
        mod.set_axon_ntff_profile_hook = lambda h: _state.__setitem__("hook", h)
        mod.get_axon_ntff_profile_hook = lambda: _state["hook"]
        sys.modules["antenv.axon_hooks"] = mod
        antenv.axon_hooks = mod
        try:
            from trn_agent_boot.trn_boot import _ntff_profile_via_ctypes
            so = "/opt/axon/libaxon_pjrt.so"
            if os.path.exists(so):
                mod.set_axon_ntff_profile_hook(_ntff_profile_via_ctypes(so))
        except Exception:
            pass
    except Exception:
        pass


# NOTE: walrus's ldw-elision pass (--enable-ldw-opt=true) rejects DoubleRow
# LDWEIGHTS ("InstLdweights is not compatible with LDW optimization"), so
# unlike the fp32r baseline we leave it off: bf16 LDWs get FWL (4-elem-wide
# loads) and shadow-load behind the previous matmul, so elision isn't needed.

_ensure_ntff_hook()

B, H, W, C, F = 32, 64, 64, 64, 128
KH = KW = 3
N_CORES = 8
IMGS = B // N_CORES          # 4 images per core
HP, WP = H + 2, W + 3        # 66 rows, 67 cols (one spare zero col)
ROWS_PER_BLK = 8
BLK = ROWS_PER_BLK * W       # 512 pixels per block
BLKS_PER_IMG = H // ROWS_PER_BLK    # 8
NBLK = IMGS * BLKS_PER_IMG   # 32 blocks per core
PIX = IMGS * H * W           # 16384 pixels per core
NTOT = B * H * W             # 131072 pixels total

F32 = mybir.dt.float32
BF16 = mybir.dt.bfloat16
FP8 = mybir.dt.float8e4
NP_BF16 = ml_dtypes.bfloat16
NP_FP8 = ml_dtypes.float8_e4m3

# q-branch fp8 tile per image: [128, 3 copies (kw shift), HQ rows, 64]
# with contiguous rows; copy c holds x[..., w+c]. HQ=67 adds a zero pad
# row so the lone-tap DoubleRow's dummy second read stays in bounds.
HQ = 67
Q_CS = HQ * W                # copy stride in elements
# DoubleRow tap pairs: (base tap, second tap or None); base tap (kh,kw)
# reads copy kw at row offset kh, the pair stride D walks to the second.
Q_PAIRS = [((0, 0), (0, 1)), ((1, 0), (0, 2)), ((1, 1), (1, 2)),
           ((2, 0), (2, 1)), ((2, 2), None)]
Q_DELTA = [Q_CS, 2 * Q_CS - W, Q_CS, Q_CS, W]
GROUPS = [(0, 1, 2, 3), (4, 5, 6, 7)]   # hb groups within an image
DR = mybir.MatmulPerfMode.DoubleRow

# epilogue add split: blocks [0, STT_SPLIT) on gpsimd (runs while DVE still
# drains conv PSUMs), rest on DVE after its drains finish
STT_SPLIT = 25

LAST_EXEC_TIME_NS = None


def _dr_rhs(xt, h0, p, nrows=HQ):
    """rhs AP [128, 2, 512] for DoubleRow pair p: base tap's 8x64 window is
    one contiguous 512-run; dim1 walks to the second tap (copy/row delta)."""
    (akh, akw), _ = Q_PAIRS[p]
    cs_ = nrows * W
    deltas = (cs_, 2 * cs_ - W, cs_, cs_, W)
    base = xt[:, akw, h0 + akh:h0 + akh + ROWS_PER_BLK, 0:W]
    raw = base.ap
    part = raw[0]
    new = [part, [deltas[p], 2], [1, ROWS_PER_BLK * W]]
    return AP(base.tensor, base.offset, new)


def _build(offset_const: float, scale_const: float, seed_recip: float,
           n_cores: int = N_CORES):
    """offset_const = 2*sum(cn)/F ; scale_const = -4/(NTOT*F).
    gamma = 1 / (offset_const + scale_const * sum_q_total); seed_recip is a
    host-side statistical estimate of gamma used only to seed Newton's
    method for the on-device reciprocal."""
    nc = bacc.Bacc("TRN2", target_bir_lowering=False, debug=False,
                   num_devices=n_cores)
    xx = nc.dram_tensor("xx", [128, IMGS, 3, HQ, W], FP8, kind="ExternalInput")
    xc = nc.dram_tensor("xc", [128, IMGS, HP, WP], BF16, kind="ExternalInput")
    qw = nc.dram_tensor("qw", [128, 5, 2, F], FP8, kind="ExternalInput")
    cwp = nc.dram_tensor("cwp", [128, 3, F], BF16, kind="ExternalInput")
    cws = nc.dram_tensor("cws", [128, 3, F], BF16, kind="ExternalInput")
    cnneg = nc.dram_tensor("cnneg", [128, 1], F32, kind="ExternalInput")
    biasf = nc.dram_tensor("biasf", [128, 1], F32, kind="ExternalInput")
    out = nc.dram_tensor("out", [128, PIX], F32, kind="ExternalOutput")

    with tile.TileContext(nc) as tc:
        with (
            tc.tile_pool(name="xp", bufs=1) as xp,
            tc.tile_pool(name="wp", bufs=1) as wp,
            tc.tile_pool(name="qs", bufs=1) as qs,
            tc.tile_pool(name="cs", bufs=30) as cs,
            tc.tile_pool(name="kn", bufs=12) as kn,
            tc.tile_pool(name="ot", bufs=6) as ot,
            tc.tile_pool(name="ps", bufs=6, space="PSUM") as ps,
            tc.tile_pool(name="pss", bufs=1, space="PSUM") as pss,
            tc.tile_pool(name="dr", bufs=1, space="DRAM") as drp,
        ):
            # ---- loads (q weights + first fp8 image first: PE starts asap)
            qwt = wp.tile([128, 5, 2, F], FP8, tag="qw")
            nc.sync.dma_start(out=qwt, in_=qw[:])
            # small head tile: rows 0..11 of image 0, so the first q group
            # starts after a 0.3MB DMA instead of the full 1.6MB image
            HHR = 12
            x8h = xp.tile([128, 3, HHR, W], FP8, tag="x8h")
            nc.sync.dma_start(out=x8h, in_=xx[:, 0, :, 0:HHR])
            x8 = []
            xcb = []
            for i in range(IMGS):
                t8 = xp.tile([128, 3, HQ, W], FP8, tag=f"x8_{i}")
                nc.sync.dma_start(out=t8, in_=xx[:, i])
                x8.append(t8)
                tb = xp.tile([128, HP, WP], BF16, tag=f"xc_{i}")
                xcb.append(tb)
            cwpt = wp.tile([128, 3, F], BF16, tag="cwp")
            nc.sync.dma_start(out=cwpt, in_=cwp[:])
            cwst = wp.tile([128, 3, F], BF16, tag="cws")
            nc.sync.dma_start(out=cwst, in_=cws[:])
            for i in range(IMGS):
                nc.sync.dma_start(out=xcb[i], in_=xc[:, i])
            cnn = wp.tile([128, 1], F32, tag="cnn")
            nc.sync.dma_start(out=cnn, in_=cnneg[:])
            bft = wp.tile([128, 1], F32, tag="bf")
            nc.sync.dma_start(out=bft, in_=biasf[:])
            ones_c = wp.tile([128, 1], F32, tag="oc")
            nc.vector.memset(ones_c, 1.0)

            qst = qs.tile([128, NBLK, BLK], BF16, tag="q")
            sq_slots = wp.tile([128, NBLK], F32, tag="sq")

            # ---- phase A: q = pc - pn/2, fp8 DoubleRow, 5 matmuls/block
            def q_group(img, grp, xt=None, nrows=HQ):
                if xt is None:
                    xt = x8[img]
                qps = [ps.tile([128, BLK], F32, tag="mm", name=f"qp{img}_{hb}")
                       for hb in grp]
                for p in range(len(Q_PAIRS)):
                    wtile = qwt[:, p]
                    for gi, hb in enumerate(grp):
                        rhs = _dr_rhs(xt, hb * ROWS_PER_BLK, p, nrows)
                        nc.tensor.matmul(qps[gi][:], wtile, rhs,
                                         start=(p == 0), stop=(p == 4),
                                         perf_mode=DR)
                for gi, hb in enumerate(grp):
                    blk = img * BLKS_PER_IMG + hb
                    nc.scalar.activation(
                        qst[:, blk, :], qps[gi][:],
                        mybir.ActivationFunctionType.Copy,
                        accum_out=sq_slots[:, blk:blk + 1],
                    )

            # img0 block 0 runs from the small head tile while the bulk of
            # the input is still in flight
            q_group(0, (0,), xt=x8h, nrows=HHR)
            q_group(0, (1, 2, 3))
            q_group(0, GROUPS[1])

            # img0 sum -> CC1 asap: its mesh setup + cross-core launch-skew
            # absorption completes while q/conv still run, so CC2 (the one
            # gating the epilogue) is a pure data-wait + mesh hops
            sq_red_a = wp.tile([128, 1], F32, tag="sqa")
            nc.vector.reduce_sum(sq_red_a, sq_slots[:, 0:8],
                                 axis=mybir.AxisListType.X)
            q_group(1, GROUPS[0])
            ps1a = pss.tile([1, 1], F32, tag="s1", name="ps1a")
            nc.tensor.matmul(ps1a[:], sq_red_a[:], ones_c[:],
                             start=True, stop=True)
            s_a = wp.tile([1, 1], F32, tag="ssa")
            nc.scalar.copy(s_a[:], ps1a[:])
            cc_in_a = drp.tile([1, 1], F32, tag="cia")
            cc_out_a = drp.tile([1, 1], F32, tag="coa")
            nc.sync.dma_start(out=cc_in_a, in_=s_a[:])
            nc.gpsimd.collective_compute(
                "AllReduce", mybir.AluOpType.add,
                replica_groups=[list(range(n_cores))],
                ins=[cc_in_a.opt()], outs=[cc_out_a.opt()],
            )

            q_group(1, GROUPS[1])
            for img in (2, 3):
                for grp in GROUPS:
                    q_group(img, grp)

            sq_red_b = wp.tile([128, 1], F32, tag="sqb")
            nc.vector.reduce_sum(sq_red_b, sq_slots[:, 8:32],
                                 axis=mybir.AxisListType.X)

            # ---- phase C+D: conv (bf16, K=128 pairs) + interleaved epilogue
            scal = wp.tile([128, 1], F32, tag="scal")
            bias_g = wp.tile([128, 1], F32, tag="bg")
            gam128 = wp.tile([128, 1], F32, tag="g128")

            cc_in_b = drp.tile([1, 1], F32, tag="cib")
            cc_out_b = drp.tile([1, 1], F32, tag="cob")

            def conv_group(img, grp):
                xt = xcb[img]
                cps = [ps.tile([128, BLK], F32, tag="mm", name=f"cp{img}_{hb}")
                       for hb in grp]
                for m in range(6):
                    if m < 3:
                        kh, c0, wtile = m, 0, cwpt[:, m]
                    else:
                        kh, c0, wtile = m - 3, 2, cwst[:, m - 3]
                    for gi, hb in enumerate(grp):
                        h0 = hb * ROWS_PER_BLK
                        rhs = xt[:, h0 + kh:h0 + kh + ROWS_PER_BLK, c0:c0 + W]
                        nc.tensor.matmul(cps[gi][:], wtile, rhs,
                                         start=(m == 0), stop=(m == 5))
                return cps

            def drain_block(cps_tile, img, hb):
                # drain conv PSUM and fold in the conv bias on DVE (Pool
                # can't read PSUM, ACT is reserved for the exps)
                blk = img * BLKS_PER_IMG + hb
                cst = cs.tile([128, BLK], BF16, tag="cst", name=f"cst{blk}")
                nc.vector.tensor_scalar(
                    out=cst[:], in0=cps_tile[:], scalar1=bft[:], scalar2=None,
                    op0=mybir.AluOpType.add)
                return blk, cst

            def ep_block(blk, cst, pend):
                kant = kn.tile([128, BLK], BF16, tag="kan", name=f"kan{blk}")
                nc.scalar.activation(
                    kant[:], qst[:, blk, :],
                    mybir.ActivationFunctionType.Exp,
                    bias=bias_g[:], scale=scal[:],
                )
                if blk < STT_SPLIT:
                    outt = ot.tile([128, BLK], F32, tag="outt",
                                   name=f"out{blk}")
                    nc.gpsimd.tensor_tensor(
                        out=outt[:], in0=kant[:], in1=cst[:],
                        op=mybir.AluOpType.add,
                    )
                    nc.sync.dma_start(out=out[:, blk * BLK:(blk + 1) * BLK],
                                      in_=outt[:])
                else:
                    pend.append((blk, cst, kant))

            pend = []
            first = True
            all_groups = [(img, grp) for img in range(IMGS)
                          for grp in GROUPS]
            for gidx, (img, grp) in enumerate(all_groups):
                cps = conv_group(img, grp)
                if first:
                    # second-half sum -> CC2 (PE already busy on conv)
                    ps1b = pss.tile([1, 1], F32, tag="s1", name="ps1b")
                    nc.tensor.matmul(ps1b[:], sq_red_b[:], ones_c[:],
                                     start=True, stop=True)
                    s_b = wp.tile([1, 1], F32, tag="ssb")
                    nc.scalar.copy(s_b[:], ps1b[:])
                    nc.sync.dma_start(out=cc_in_b, in_=s_b[:])
                    nc.gpsimd.collective_compute(
                        "AllReduce", mybir.AluOpType.add,
                        replica_groups=[list(range(n_cores))],
                        ins=[cc_in_b.opt()], outs=[cc_out_b.opt()],
                    )
                    stot_a = wp.tile([1, 1], F32, tag="sta")
                    nc.sync.dma_start(out=stot_a, in_=cc_out_a)
                    stot_b = wp.tile([1, 1], F32, tag="stb")
                    nc.sync.dma_start(out=stot_b, in_=cc_out_b)

                    # gamma = 1/den via exp(-ln(den)) entirely on ACT ([1,1]
                    # ops are ~0.3us there, vs ~1.3us/op on the Pool ucode
                    # path); only the 128-partition broadcast runs on Pool,
                    # whose queue holds nothing but the collective triggers.
                    t_ = wp.tile([1, 1], F32, tag="gt")
                    nc.scalar.activation(
                        t_[:], stot_a[:], mybir.ActivationFunctionType.Copy,
                        bias=float(offset_const), scale=float(scale_const))
                    den = wp.tile([1, 1], F32, tag="den")
                    nc.scalar.activation(
                        den[:], stot_b[:], mybir.ActivationFunctionType.Relu,
                        bias=t_[:], scale=float(scale_const))
                    lnd = wp.tile([1, 1], F32, tag="lnd")
                    nc.scalar.activation(
                        lnd[:], den[:], mybir.ActivationFunctionType.Ln)
                    gam = wp.tile([1, 1], F32, tag="gam")
                    nc.scalar.activation(
                        gam[:], lnd[:], mybir.ActivationFunctionType.Exp,
                        scale=-1.0)
                    nc.gpsimd.partition_broadcast(gam128[:], gam[:])
                    nc.scalar.activation(
                        scal[:], gam128[:], mybir.ActivationFunctionType.Copy,
                        scale=2.0)
                    nc.gpsimd.tensor_tensor(
                        out=bias_g[:], in0=gam128[:], in1=cnn[:],
                        op=mybir.AluOpType.mult)
                    first = False
                for gi, hb in enumerate(grp):
                    blk, cst = drain_block(cps[gi], img, hb)
                    ep_block(blk, cst, pend)

            for blk, cst, kant in pend:
                outt = ot.tile([128, BLK], F32, tag="outt", name=f"out{blk}")
                nc.vector.tensor_tensor(
                    out=outt[:], in0=kant[:], in1=cst[:],
                    op=mybir.AluOpType.add,
                )
                nc.sync.dma_start(out=out[:, blk * BLK:(blk + 1) * BLK],
                                  in_=outt[:])

    nc.compile()
    return nc


def _prep_inputs(inputs, kernel, bias, control_points):
    x = np.ascontiguousarray(np.asarray(inputs, dtype=np.float32))
    kw_ = np.asarray(kernel, dtype=np.float32)
    bias = np.asarray(bias, dtype=np.float32)
    cp = np.asarray(control_points, dtype=np.float32)

    # q weights: DoubleRow pairs [c, pair, i, f]; rows 64..127 hit x^2
    qw = np.zeros((128, 5, 2, F), dtype=NP_FP8)
    for p, (a, b) in enumerate(Q_PAIRS):
        for i, t in enumerate((a, b)):
            if t is None:
                continue
            qw[0:C, p, i, :] = cp[t[0], t[1]].astype(NP_FP8)
            qw[C:128, p, i, :] = NP_FP8(-0.5)

    # conv weights: pairs [(kh,0);(kh,1)] and zero-padded singles [(kh,2);0]
    cwp = np.zeros((128, 3, F), dtype=NP_BF16)
    cws = np.zeros((128, 3, F), dtype=NP_BF16)
    for kh in range(KH):
        cwp[0:C, kh, :] = kw_[kh, 0].astype(NP_BF16)
        cwp[C:128, kh, :] = kw_[kh, 1].astype(NP_BF16)
        cws[0:C, kh, :] = kw_[kh, 2].astype(NP_BF16)

    cn = (cp.reshape(KH * KW * C, F).astype(np.float64) ** 2).sum(axis=0)
    offset_const = float(2.0 * cn.sum() / F)
    scale_const = float(-4.0 / (NTOT * F))
    cnneg = np.ascontiguousarray(-cn.astype(np.float32).reshape(F, 1))
    biasf = np.ascontiguousarray(bias.reshape(F, 1))

    in_maps = []
    for core in range(N_CORES):
        xs = x[core * IMGS:(core + 1) * IMGS]          # [4,64,64,64]
        xt = xs.transpose(3, 0, 1, 2)                  # [C,4,64,64]
        xpad = np.zeros((C, IMGS, HP, WP), np.float32)
        xpad[:, :, 1:H + 1, 1:W + 1] = xt
        # fp8 [x | x^2], three column-shifted copies with row stride W
        xx8 = np.zeros((128, IMGS, 3, HQ, W), dtype=NP_FP8)
        xsq = xpad * xpad
        for kwi in range(3):
            sl = xpad[:, :, :, kwi:kwi + W]          # [C, IMGS, HP, W]
            sq = xsq[:, :, :, kwi:kwi + W]
            xx8[0:C, :, kwi, 0:HP, :] = sl.transpose(0, 1, 2, 3).astype(NP_FP8)
            xx8[C:128, :, kwi, 0:HP, :] = sq.astype(NP_FP8)
        # bf16 [x | x shifted left one column]
        xcb = np.zeros((128, IMGS, HP, WP), dtype=NP_BF16)
        xcb[0:C] = xpad.astype(NP_BF16)
        xcb[C:128, :, :, 0:WP - 1] = xpad[:, :, :, 1:WP].astype(NP_BF16)
        in_maps.append({
            "xx": np.ascontiguousarray(xx8),
            "xc": np.ascontiguousarray(xcb),
            "qw": qw, "cwp": cwp, "cws": cws,
            "cnneg": cnneg, "biasf": biasf,
        })
    # statistical seed for the on-device Newton reciprocal of den:
    # den ~= offset + scale * (-(9C/2)*mean(x^2)*NTOT*F); NR tolerates ~2x
    s_est = -0.5 * KH * KW * C * float(np.mean(x * x)) * NTOT * F
    seed_recip = 1.0 / (offset_const + scale_const * s_est)
    return in_maps, offset_const, scale_const, seed_recip


def kernel(inputs, kernel, bias, control_points):
    global LAST_EXEC_TIME_NS
    in_maps, offset_const, scale_const, seed_recip = _prep_inputs(
        inputs, kernel, bias, control_points)

    nc = _build(offset_const, scale_const, seed_recip)
    res = run_bass_kernel_spmd(nc, in_maps, core_ids=list(range(N_CORES)))
    LAST_EXEC_TIME_NS = res.exec_time_ns

    out = np.empty((B, H, W, F), np.float32)
    for core in range(N_CORES):
        o = res.results[core]["out"]                   # [128, PIX]
        o = o.reshape(F, IMGS, H, W).transpose(1, 2, 3, 0)
        out[core * IMGS:(core + 1) * IMGS] = o
    return out
